# revision 9
# baseline (speedup 1.0000x reference)
"""Trainium2 Bass kernel for GCNBlock (spectral-norm linear + GCN aggregation +
InstanceNorm + LeakyReLU) distributed across 8 NeuronCores.

v2 — DMA-roofline oriented rewrite of the gather/scatter GCN kernel.

Strategy (dst-sharded, as v1):
  - out = (A @ xs) @ WnT with xs = dinv*x host-prescaled; per-dst dinv folded
    into eps' = deg*eps (and bias' = sqrt(deg)*b when b != 0), so scatter
    matrices are pure one-hot.
  - dst nodes sharded across 8 cores (49 tiles of 128 per core); edges
    partitioned by (tile, src-half) into 128-edge blocks; per block a SWDGE
    dma_gather pulls the 128 source rows (bf16) and the PE accumulates
    aggT[cin, dst] += Xsrc.T @ S in PSUM.

v2 changes (driven by the v1 trace: 16 SDMA engines ~each 158us busy out of a
328us kernel; HWDGE S-matrix stream = 13.6MB/core of that; 70us un-overlapped
tail; PE at HAM half-rate):
  - One-hot S is built ON DEVICE by the (otherwise idle) DVE: one
    tensor_tensor(is_equal) per chunk with stride-0 broadcast APs compares a
    streamed per-slot dstloc value (bf16, [128, totg] = 0.2MB vs 13.6MB) with
    a constant iota row. Removes ~2.2k large HWDGE packets/core.
  - Self-loop rows are folded in as matmul(lhsT=Identity, rhs=xsT_slice) from
    an SBUF-resident feature-major copy of this core's own xs slice (one
    1.6MB HWDGE load in 128 big descriptors, vs 6.3k small per-tile loads).
    The identity tile is built on device (is_equal of iota row vs column).
  - The weight/norm tail is interleaved per-tile/per-batch into the hot loop:
    pass-1 (po = aggT.T@WnT_ext with a 129th mean column; ACT Square accum →
    ssq, ACT copy → mu) runs right after each tile's aggregation drains, and
    every 7 tiles the batched norm scalars (DVE) + gated pass-2 (PE matmul +
    fused ACT Prelu(po*rstd - mu*rstd)) + output DMA follow.  The PE stays
    continuously busy (HAM stays at full rate) and the tail vanishes into the
    DMA-bound hot loop.
  - agg_all kept in bf16 (halves PSUM-drain writes and pass LDWEIGHTS time).
  - Output written as bf16 (halves output packets); host upcasts to fp32.
"""

import numpy as np
import ml_dtypes
from contextlib import ExitStack

import concourse.tile as tile
from concourse import bacc, mybir
from concourse.bass_utils import run_bass_kernel_spmd

_N0 = None

# Problem constants (hardcoded per spec)
N, E, C = 50000, 800000, 128
P = 128
NCORES = 8
TPC = 49                # dst tiles per core
NPC = TPC * P           # 6272 dst nodes per core
NPAD = NCORES * NPC     # 50176 padded node count
HALF = 32768            # int16 index window size
HALF2 = NPAD - HALF     # hi-window base (17408); [HALF2, HALF) rows are flexible
CHUNK_TILES = 3
NCHUNKS = -(-TPC // CHUNK_TILES)  # 17 (last chunk ragged)
NQ = 4                  # SWDGE queues (Q7 core pairs) used for gathers
EPS_IN = 1e-5
CW = C + 1              # weight matmul width (extra column = row mean)
NB = 7                  # tiles per norm/pass-2 batch


def _preprocess(x, edge_index, W, b, u):
    """Host-side prep: spectral norm, edge partitioning, metadata layout."""
    x = np.asarray(x, dtype=np.float32)
    ei = np.asarray(edge_index)
    W = np.asarray(W, dtype=np.float32)
    b = np.asarray(b, dtype=np.float32)
    u = np.asarray(u, dtype=np.float32)

    # --- spectral norm (one power iteration), matches reference ---
    eps = np.float32(1e-12)
    v = (W.T @ u).astype(np.float32)
    v = v / (np.float32(np.linalg.norm(v)) + eps)
    Wv = (W @ v).astype(np.float32)
    u2 = Wv / (np.float32(np.linalg.norm(Wv)) + eps)
    sigma = np.float32(u2 @ Wv)
    WnT = np.ascontiguousarray((W / sigma).T, dtype=np.float32)  # [cin, cout]
    # extended weight: col C = row-mean column (mu comes out of the matmul)
    WnT_ext = np.concatenate([WnT, WnT.mean(axis=1, keepdims=True)], axis=1)
    WnT_ext = np.ascontiguousarray(WnT_ext.astype(ml_dtypes.bfloat16))
    b_ext = np.concatenate([b, [b.mean()]]).reshape(1, CW).astype(ml_dtypes.bfloat16)
    b_nonzero = bool(np.any(b))

    src = ei[0].astype(np.int64)
    dst = ei[1].astype(np.int64)

    # --- degrees; xs = dinv * x (row-scaled source features) ---
    deg = (np.bincount(dst, minlength=N) + 1).astype(np.float32)
    dinv = (1.0 / np.sqrt(deg)).astype(np.float32)
    deg_pad = np.ones(NPAD, dtype=np.float32)
    deg_pad[:N] = deg
    sqrtdeg_pad = np.sqrt(deg_pad)

    # --- group real edges by (core, tile, src-window) ---
    # The lo window covers src [0, HALF); the hi window covers [HALF2, NPAD).
    # src in [HALF2, HALF) can go to either group: per (core, tile) we pick the
    # split so both groups pack into a shared minimal number of 128-edge blocks.
    core = dst // NPC
    tile_g = (dst % NPC) // P
    dstloc = (dst % P).astype(np.float32)
    grp = core * TPC + tile_g
    NGT = NCORES * TPC
    total_ct = np.bincount(grp, minlength=NGT).reshape(NCORES, TPC)
    nlo_fix = np.bincount(grp[src < HALF2], minlength=NGT).reshape(NCORES, TPC)
    nhi_fix = np.bincount(grp[src >= HALF], minlength=NGT).reshape(NCORES, TPC)
    nflex = total_ct - nlo_fix - nhi_fix
    B_t = np.ceil(total_ct.max(axis=0) / P).astype(np.int64)            # [TPC]
    nb_lo = np.ceil(nlo_fix.max(axis=0) / P).astype(np.int64)
    nb_hi = np.maximum(np.ceil(nhi_fix.max(axis=0) / P).astype(np.int64),
                       B_t - nb_lo)
    # flex edges assigned to lo per (core, tile)
    k_ct = np.clip(nflex + nhi_fix - nb_hi[None, :] * P, 0, nflex)
    nlo_cnt = nlo_fix + k_ct
    assert (nlo_cnt <= nb_lo[None, :] * P).all()
    assert (total_ct - nlo_cnt <= nb_hi[None, :] * P).all()
    # src-sorted rank within (core, tile): first nlo_cnt edges -> lo window
    order0 = np.lexsort((src, grp))
    starts0 = np.zeros(NGT + 1, dtype=np.int64)
    np.cumsum(np.bincount(grp, minlength=NGT), out=starts0[1:])
    rank0 = np.arange(len(grp), dtype=np.int64) - starts0[grp[order0]]
    half = np.empty(len(grp), dtype=np.int64)
    half[order0] = (rank0 >= nlo_cnt.reshape(-1)[grp[order0]]).astype(np.int64)

    key = (grp * 2 + half).astype(np.int64)
    NG = NCORES * TPC * 2
    order = np.argsort(key, kind="stable")
    counts = np.bincount(key, minlength=NG)
    starts = np.zeros(NG + 1, dtype=np.int64)
    np.cumsum(counts, out=starts[1:])
    rank = np.arange(len(key), dtype=np.int64) - starts[key[order]]

    nb = np.stack([nb_lo, nb_hi], axis=1)  # [TPC, 2] gather blocks

    # Gather-column layout per chunk: [lo blocks | hi blocks] (no self cols).
    blk_gcol = np.zeros((TPC, 2), dtype=np.int64)  # global gather column of run
    gather_gcol0 = np.zeros((NCHUNKS, 2), dtype=np.int64)
    gather_nblk = np.zeros((NCHUNKS, 2), dtype=np.int64)
    gpos = 0
    for ci in range(NCHUNKS):
        t0 = ci * CHUNK_TILES
        t1 = min(t0 + CHUNK_TILES, TPC)
        for h in range(2):
            gather_gcol0[ci, h] = gpos
            for t in range(t0, t1):
                blk_gcol[t, h] = gpos
                gpos += nb[t, h]
            gather_nblk[ci, h] = gpos - gather_gcol0[ci, h]
    totg = gpos

    # per-slot dst-local value per gather column (-1 for pad slots -> zero row)
    DSTLOC = np.full((NCORES, P, totg), -1.0, dtype=np.float32)
    IDXALL = np.zeros((NCORES, totg * P), dtype=np.int16)

    o_core = core[order]
    o_tile = tile_g[order]
    o_half = half[order]
    o_gcol = blk_gcol[o_tile, o_half] + rank // P
    o_slot = rank % P

    DSTLOC[o_core, o_slot, o_gcol] = dstloc[order]
    IDXALL[o_core, o_gcol * P + o_slot] = (src[order] - o_half * HALF2).astype(np.int16)
    DSTLOC = DSTLOC.astype(ml_dtypes.bfloat16)

    # idx SBUF layout: pos k -> [k % 16, k // 16], replicated 8x over partitions
    IDX = np.tile(IDXALL.reshape(NCORES, -1, 16).transpose(0, 2, 1), (1, 8, 1))
    n0 = int(gather_nblk[0].sum())  # chunk-0 gather blocks (loaded first)

    xs_pad = np.zeros((NPAD, C), dtype=ml_dtypes.bfloat16)
    xs_pad[:N] = (dinv[:, None] * x).astype(ml_dtypes.bfloat16)
    # feature-major self rows: XST[core][c, d_local] (bf16)
    XST = np.ascontiguousarray(
        xs_pad.reshape(NCORES, NPC, C).transpose(0, 2, 1)
    )

    # iota helper: cols 0..127 = column index, col 128 = partition index
    IOTA = np.zeros((P, P + 1), dtype=ml_dtypes.bfloat16)
    IOTA[:, :P] = np.arange(P, dtype=np.float32)[None, :].astype(ml_dtypes.bfloat16)
    IOTA[:, P] = np.arange(P, dtype=np.float32).astype(ml_dtypes.bfloat16)
    IOTAC = np.arange(P, dtype=np.float32).reshape(P, 1)  # fp32 partition index

    SQRTDEG = sqrtdeg_pad.reshape(NCORES, 1, NPC).astype(ml_dtypes.bfloat16)
    EPSDEG = (EPS_IN * deg_pad).reshape(NCORES, TPC, P).transpose(0, 2, 1)
    EPSDEG = np.ascontiguousarray(EPSDEG, dtype=np.float32)  # [NCORES, P, TPC]

    meta = dict(
        nb=nb,
        blk_gcol=blk_gcol,
        gather_gcol0=gather_gcol0,
        gather_nblk=gather_nblk,
        totg=totg,
        n0=n0,
        b_nonzero=b_nonzero,
    )
    global _N0
    _N0 = n0
    return xs_pad, XST, IDX, DSTLOC, IOTA, IOTAC, SQRTDEG, EPSDEG, WnT_ext, b_ext, meta


def _build(meta):
    """Build the SPMD Bass graph (shared across all 8 cores)."""
    nb = meta["nb"]
    b_nonzero = meta["b_nonzero"]
    blk_gcol = meta["blk_gcol"]
    gather_gcol0 = meta["gather_gcol0"]
    gather_nblk = meta["gather_nblk"]
    totg = meta["totg"]

    nc = bacc.Bacc(
        "TRN2", target_bir_lowering=False, debug=False, num_swdge_queues=NQ
    )

    x_d = nc.dram_tensor("x", [NPAD, C], mybir.dt.bfloat16, kind="ExternalInput")
    xst_d = nc.dram_tensor("xst", [C, NPC], mybir.dt.bfloat16, kind="ExternalInput")
    n0 = meta["n0"]
    idx0_d = nc.dram_tensor("idx0", [P, n0 * 8], mybir.dt.int16, kind="ExternalInput")
    idx_d = nc.dram_tensor("idx", [P, totg * 8], mybir.dt.int16, kind="ExternalInput")
    dstloc_d = nc.dram_tensor("dstloc", [P, totg], mybir.dt.bfloat16, kind="ExternalInput")
    iota_d = nc.dram_tensor("iota", [P, P + 1], mybir.dt.bfloat16, kind="ExternalInput")
    iotac_d = nc.dram_tensor("iotac", [P, 1], mybir.dt.float32, kind="ExternalInput")
    sd_d = nc.dram_tensor("sqrtdeg", [1, NPC], mybir.dt.bfloat16, kind="ExternalInput")
    epsdeg_d = nc.dram_tensor("epsdeg", [P, TPC], mybir.dt.float32, kind="ExternalInput")
    wnT_d = nc.dram_tensor("wnT", [C, CW], mybir.dt.bfloat16, kind="ExternalInput")
    b_d = nc.dram_tensor("b", [1, CW], mybir.dt.bfloat16, kind="ExternalInput")
    out_d = nc.dram_tensor("out", [NPC, C], mybir.dt.bfloat16, kind="ExternalOutput")

    # max gather blocks per chunk for each half (separate tiles per half)
    nlo_max = max(int(gather_nblk[ci, 0]) for ci in range(NCHUNKS))
    nhi_max = max(int(gather_nblk[ci, 1]) for ci in range(NCHUNKS))
    nblk_max = max(int(gather_nblk[ci].sum()) for ci in range(NCHUNKS))

    qctr = 0  # gather round-robin queue counter

    with tile.TileContext(nc) as tc, ExitStack() as ctx:
        meta_p = ctx.enter_context(tc.tile_pool(name="meta", bufs=1))
        gat_p = ctx.enter_context(tc.tile_pool(name="gat", bufs=7))
        s_p = ctx.enter_context(tc.tile_pool(name="s", bufs=4))
        out_p = ctx.enter_context(tc.tile_pool(name="out", bufs=6))
        small_p = ctx.enter_context(tc.tile_pool(name="small", bufs=1))
        ps_p = ctx.enter_context(tc.tile_pool(name="ps", bufs=8, space="PSUM"))

        idxz = meta_p.tile([P, 8], mybir.dt.int16)
        nc.vector.memset(idxz[:], 0)
        warm = meta_p.tile([P, 1, P], mybir.dt.bfloat16)
        nc.gpsimd.dma_gather(
            out_ap=warm[:], in_ap=x_d[0:HALF, :], idxs_ap=idxz[:, 0:8],
            num_idxs=P, num_idxs_reg=P, elem_size=C,
            single_packet=False, queue_num=0,
        )
        idx0_sb = meta_p.tile([P, n0 * 8], mybir.dt.int16)
        nc.sync.dma_start(idx0_sb[:], idx0_d[:])
        idx_sb = meta_p.tile([P, totg * 8], mybir.dt.int16)
        nc.sync.dma_start(idx_sb[:], idx_d[:])
        dstloc_sb = meta_p.tile([P, totg], mybir.dt.bfloat16)
        nc.sync.dma_start(dstloc_sb[:], dstloc_d[:])
        iota_sb = meta_p.tile([P, P + 1], mybir.dt.bfloat16)
        nc.sync.dma_start(iota_sb[:], iota_d[:])
        iotac_sb = meta_p.tile([P, 1], mybir.dt.float32)
        nc.sync.dma_start(iotac_sb[:], iotac_d[:])
        xst_sb = meta_p.tile([C, NPC], mybir.dt.bfloat16)
        nc.sync.dma_start(xst_sb[:], xst_d[:])
        epsdeg_sb = meta_p.tile([P, TPC], mybir.dt.float32)
        nc.sync.dma_start(epsdeg_sb[:], epsdeg_d[:])
        wnT_sb = meta_p.tile([C, CW], mybir.dt.bfloat16)
        nc.sync.dma_start(wnT_sb[:], wnT_d[:])
        if b_nonzero:
            sd_sb = meta_p.tile([1, NPC], mybir.dt.bfloat16)
            nc.sync.dma_start(sd_sb[:], sd_d[:])
            b_sb = meta_p.tile([1, CW], mybir.dt.bfloat16)
            nc.sync.dma_start(b_sb[:], b_d[:])

        # identity (bf16) for the self-row matmuls: I[p, j] = (j == p)
        ident = meta_p.tile([P, P], mybir.dt.bfloat16)
        nc.vector.tensor_scalar(
            out=ident[:], in0=iota_sb[:, 0:P], scalar1=iotac_sb[:],
            scalar2=None, op0=mybir.AluOpType.is_equal,
        )

        # persistent aggregation output (bf16) + per-tile norm stats
        agg_all = meta_p.tile([P, TPC * C], mybir.dt.bfloat16)
        mu_sb = meta_p.tile([P, TPC], mybir.dt.float32)
        ssq_sb = meta_p.tile([P, TPC], mybir.dt.float32)

        x_lo = x_d[0:HALF, :]
        x_hi = x_d[HALF2 : HALF2 + HALF, :]

        def batch_tail(g):
            """Norm scalars + gated pass-2 + output DMA for tile batch g."""
            tb0, tb1 = g * NB, min((g + 1) * NB, TPC)
            nt_b = tb1 - tb0
            mu_ap = mu_sb[:, tb0:tb1]
            msq = small_p.tile([P, nt_b], mybir.dt.float32, tag=f"msq{g}")
            nc.vector.tensor_tensor(out=msq[:], in0=mu_ap, in1=mu_ap, op=mybir.AluOpType.mult)
            var = small_p.tile([P, nt_b], mybir.dt.float32, tag=f"var{g}")
            nc.vector.tensor_scalar(
                out=var[:], in0=ssq_sb[:, tb0:tb1], scalar1=1.0 / C, scalar2=None,
                op0=mybir.AluOpType.mult,
            )
            var2 = small_p.tile([P, nt_b], mybir.dt.float32, tag=f"var2{g}")
            nc.vector.tensor_tensor(out=var2[:], in0=var[:], in1=msq[:], op=mybir.AluOpType.subtract)
            var3 = small_p.tile([P, nt_b], mybir.dt.float32, tag=f"var3{g}")
            nc.vector.tensor_tensor(
                out=var3[:], in0=var2[:], in1=epsdeg_sb[:, tb0:tb1], op=mybir.AluOpType.add
            )
            std = small_p.tile([P, nt_b], mybir.dt.float32, tag=f"std{g}")
            nc.scalar.activation(
                out=std[:], in_=var3[:], func=mybir.ActivationFunctionType.Sqrt,
            )
            rstd = small_p.tile([P, nt_b], mybir.dt.float32, tag=f"rstd{g}")
            nc.vector.reciprocal(out=rstd[:], in_=std[:])
            nmr0 = small_p.tile([P, nt_b], mybir.dt.float32, tag=f"nmr0{g}")
            nc.vector.tensor_tensor(out=nmr0[:], in0=mu_ap, in1=rstd[:], op=mybir.AluOpType.mult)
            nmr = small_p.tile([P, nt_b], mybir.dt.float32, tag=f"nmr{g}")
            nc.vector.tensor_scalar(
                out=nmr[:], in0=nmr0[:], scalar1=-1.0, scalar2=None,
                op0=mybir.AluOpType.mult,
            )

            # gate this batch's pass-2 matmuls on its rstd so their PSUM tiles
            # cannot pin slots before the norm scalars exist
            zcol = small_p.tile([P, 1], mybir.dt.float32, tag=f"zcol{g}")
            nc.vector.tensor_scalar(
                out=zcol[:], in0=rstd[:, 0:1], scalar1=0.0, scalar2=None,
                op0=mybir.AluOpType.mult,
            )
            wnT2_sb = meta_p.tile([C, CW], mybir.dt.bfloat16, tag=f"wnT2{g}")
            nc.scalar.activation(
                out=wnT2_sb[:], in_=wnT_sb[:],
                func=mybir.ActivationFunctionType.Identity, bias=zcol[0:C, 0:1], scale=1.0,
            )
            for t in range(tb0, tb1):
                po2 = ps_p.tile([P, CW], mybir.dt.float32, tag="ps")
                nc.tensor.matmul(
                    po2[:], lhsT=agg_all[:, t * C : (t + 1) * C], rhs=wnT2_sb[:],
                    start=True, stop=not b_nonzero,
                )
                if b_nonzero:
                    nc.tensor.matmul(
                        po2[:], lhsT=sd_sb[:, t * P : (t + 1) * P], rhs=b_sb[:],
                        start=False, stop=True,
                    )
                final = out_p.tile([P, P], mybir.dt.bfloat16, tag="final")
                # fused normalize + LeakyReLU: Prelu(po*rstd - mu*rstd, alpha=0.2)
                nc.scalar.activation(
                    out=final[:], in_=po2[:, 0:C],
                    func=mybir.ActivationFunctionType.Prelu,
                    bias=nmr[:, t - tb0 : t - tb0 + 1], scale=rstd[:, t - tb0 : t - tb0 + 1], alpha=0.2,
                )
                nc.sync.dma_start(out_d[t * P : (t + 1) * P, :], final[:])

        for ci in range(NCHUNKS):
            t0 = ci * CHUNK_TILES
            t1 = min(t0 + CHUNK_TILES, TPC)
            g0 = int(gather_gcol0[ci, 0])
            nblk_ci = int(gather_nblk[ci].sum())

            gat_lo = gat_p.tile([P, nlo_max, P], mybir.dt.bfloat16, tag="glo")
            gat_hi = gat_p.tile([P, nhi_max, P], mybir.dt.bfloat16, tag="ghi")
            gat_half = [gat_lo, gat_hi]

            # gathers: separate dst tiles per half -> no WAW between them, so
            # up to 4 gathers (2 chunks x 2 halves) run on 4 Q7 pairs at once.
            # Alternate half order per chunk so queue round-robin spreads the
            # (larger) lo and (smaller) hi calls evenly.
            halves = ((0, x_lo), (1, x_hi)) if ci % 2 == 0 else ((1, x_hi), (0, x_lo))
            for h, src_ap in halves:
                nblk_g = int(gather_nblk[ci, h])
                if nblk_g == 0:
                    continue
                # split into two sub-gathers on different queues; rotate the
                # queue offset per chunk so lo/hi sizes balance across queues
                nb1 = (nblk_g + 1) // 2
                for b0, b1 in ((0, nb1), (nb1, nblk_g)):
                    if b1 <= b0:
                        continue
                    nidx = (b1 - b0) * P
                    ic0 = (int(gather_gcol0[ci, h]) + b0) * 8
                    idx_src = idx0_sb if ci == 0 else idx_sb
                    nc.gpsimd.dma_gather(
                        out_ap=gat_half[h][:, b0:b1, :],
                        in_ap=src_ap,
                        idxs_ap=idx_src[:, ic0 : ic0 + nidx // 16],
                        num_idxs=nidx,
                        num_idxs_reg=nidx,
                        elem_size=C,
                        single_packet=False,
                        queue_num=(qctr + ci) % NQ,
                    )
                    qctr += 1

            # one-hot S for the whole chunk in one DVE op:
            # S[slot, blk, dst] = (dstloc[slot, g0+blk] == dst)
            s_sb = s_p.tile([P, nblk_max, P], mybir.dt.float8e4, tag="sblk")
            nc.vector.tensor_tensor(
                out=s_sb[:, 0:nblk_ci, :],
                in0=dstloc_sb[:, g0 : g0 + nblk_ci].unsqueeze(2).broadcast_to([P, nblk_ci, P]),
                in1=iota_sb[:, 0:P].unsqueeze(1).broadcast_to([P, nblk_ci, P]),
                op=mybir.AluOpType.is_equal,
            )

            for t in range(t0, t1):
                # (source tile, source column, S column) per gather block
                blocks = []
                for h in range(2):
                    loc0 = int(blk_gcol[t, h]) - int(gather_gcol0[ci, h])
                    scol0 = int(blk_gcol[t, h]) - g0
                    for j in range(int(nb[t, h])):
                        blocks.append((gat_half[h], loc0 + j, scol0 + j))

                pt = ps_p.tile([P, C], mybir.dt.float32, tag="ps")
                # self rows first: aggT += I.T @ xsT_slice
                nc.tensor.matmul(
                    pt[:], lhsT=ident[:], rhs=xst_sb[:, t * P : (t + 1) * P],
                    start=True, stop=False,
                )
                for j, (gtile, gcol, scol) in enumerate(blocks):
                    nc.tensor.matmul(
                        pt[:],
                        lhsT=gtile[:, gcol, :],
                        rhs=s_sb[:, scol, :],
                        start=False,
                        stop=(j == len(blocks) - 1),
                    )

                nc.scalar.copy(agg_all[:, t * C : (t + 1) * C], pt[:])

                # ---- pass 1 for this tile (PE + ACT only) ----
                po = ps_p.tile([P, CW], mybir.dt.float32, tag="ps")
                nc.tensor.matmul(
                    po[:], lhsT=agg_all[:, t * C : (t + 1) * C], rhs=wnT_sb[:],
                    start=True, stop=not b_nonzero,
                )
                if b_nonzero:
                    # bias' = sqrt(deg) * b (per-dst row scale folded into lhsT)
                    nc.tensor.matmul(
                        po[:], lhsT=sd_sb[:, t * P : (t + 1) * P], rhs=b_sb[:],
                        start=False, stop=True,
                    )
                sqj = small_p.tile([P, P], mybir.dt.bfloat16, tag="sqj")
                nc.scalar.activation(
                    out=sqj[:], in_=po[:, 0:C],
                    func=mybir.ActivationFunctionType.Square,
                    accum_out=ssq_sb[:, t : t + 1],
                )
                nc.scalar.copy(mu_sb[:, t : t + 1], po[:, C : C + 1])

                # ---- batched norm + pass 2 once a batch completes ----
                if (t + 1) % NB == 0:
                    batch_tail(t // NB)
        if TPC % NB != 0:
            batch_tail(TPC // NB)

    nc.compile()
    return nc


_N0 = None


def _make_in_maps(xs_pad, XST, IDX, DSTLOC, IOTA, IOTAC, SQRTDEG, EPSDEG, WnT_ext, b_ext):
    return [
        {
            "x": xs_pad,
            "xst": np.ascontiguousarray(XST[i]),
            "idx": np.ascontiguousarray(IDX[i]),
            "idx0": np.ascontiguousarray(IDX[i][:, : _N0 * 8]),
            "dstloc": np.ascontiguousarray(DSTLOC[i]),
            "iota": IOTA,
            "iotac": IOTAC,
            "sqrtdeg": np.ascontiguousarray(SQRTDEG[i]),
            "epsdeg": np.ascontiguousarray(EPSDEG[i]),
            "wnT": WnT_ext,
            "b": b_ext,
        }
        for i in range(NCORES)
    ]


def kernel(x, edge_index, W, b, u):
    pre = _preprocess(x, edge_index, W, b, u)
    nc = _build(pre[-1])
    in_maps = _make_in_maps(*pre[:-1])

    # The axon terminal can be transiently unavailable right after a prior
    # process's teardown; retry with backoff.
    import time

    last_err = None
    for attempt in range(6):
        try:
            res = run_bass_kernel_spmd(nc, in_maps, list(range(NCORES)))
            break
        except Exception as e:  # noqa: BLE001
            last_err = e
            time.sleep(45)
    else:
        raise last_err
    shards = [np.asarray(res.results[i]["out"]) for i in range(NCORES)]
    out = np.concatenate(shards, axis=0)[:N]
    return out.astype(np.float32)


# revision 11
# speedup vs baseline: 1.4547x; 1.4547x over previous
"""Trainium2 Bass kernel for GCNBlock (spectral-norm linear + GCN aggregation +
InstanceNorm + LeakyReLU) distributed across 8 NeuronCores.

v3 — single-pass, DVE-free hot loop.

Strategy (dst-sharded):
  - out = (A @ xs) @ WnT with xs = dinv*x host-prescaled; per-dst dinv folded
    into eps' = deg*eps (and bias' = sqrt(deg)*b when b != 0), so scatter
    matrices are pure one-hot (host-precomputed fp8, streamed per chunk at
    line rate).
  - dst nodes sharded across 8 cores (49 tiles of 128 per core); edges
    partitioned by (tile, src-half) into 128-edge blocks; per block a SWDGE
    dma_gather pulls the 128 source rows (bf16) and the PE accumulates
    aggT[cin, dst] += Xsrc.T @ S in PSUM across 4 SWDGE queues.

Learned constraints driving this shape:
  - DVE is OFF LIMITS in the hot loop: the Q7 SWDGE desc-gen shares DVE's
    SBUF port pair, and any in-loop DVE op serializes against the gathers
    (measured: [128,7] DVE ops at 8.5us during desc-gen; gathers slow too).
    All elementwise work is therefore ACT-only, restructured as unary
    activation(func, scale, bias) chains with per-partition [P,1] APs.
  - 16 SDMA engines' per-packet dwell is the roofline: v1 spent 66us/engine
    on HWDGE packets.  v3 cuts HWDGE bytes: self rows come from one resident
    feature-major xsT load (128 big descs, folded into the PSUM chain as
    matmul(lhsT=I, rhs=xsT_slice)) instead of 6.3k small loads; output is
    bf16 (host upcasts); weights/aux are bf16.
  - The tail is gone: everything streams per tile inside the hot loop.
    After a tile's aggregation chain drains (ACT copy pt->aggT scratch), one
    PE matmul forms po = aggT.T @ WnT_ext (129th col = row mean) in PSUM,
    then an ACT chain computes Square+accum -> ssq, negmu, mu^2, t2 =
    ssq/C + deg*eps, rstd = Rsqrt(-mu^2 + t2), nmr = negmu*rstd, and the
    fused Prelu(po*rstd + nmr) -> bf16 out, with po read directly from PSUM
    (no second weight matmul, no persistent agg_all).  The PE stays
    continuously busy so HAM reaches full rate.
"""

import numpy as np
import ml_dtypes
from contextlib import ExitStack

import concourse.tile as tile
from concourse import bacc, mybir
from concourse.bass_utils import run_bass_kernel_spmd

_N0 = None

# Problem constants (hardcoded per spec)
N, E, C = 50000, 800000, 128
P = 128
NCORES = 8
TPC = 49                # dst tiles per core
NPC = TPC * P           # 6272 dst nodes per core
NPAD = NCORES * NPC     # 50176 padded node count
HALF = 32768            # int16 index window size
HALF2 = NPAD - HALF     # hi-window base (17408); [HALF2, HALF) rows are flexible
CHUNK_TILES = 3
NCHUNKS = -(-TPC // CHUNK_TILES)  # 17 (last chunk ragged)
NQ = 4                  # SWDGE queues (Q7 core pairs) used for gathers
EPS_IN = 1e-5
CW = C + 1              # weight matmul width (extra column = row mean)


def _preprocess(x, edge_index, W, b, u):
    """Host-side prep: spectral norm, edge partitioning, metadata layout."""
    x = np.asarray(x, dtype=np.float32)
    ei = np.asarray(edge_index)
    W = np.asarray(W, dtype=np.float32)
    b = np.asarray(b, dtype=np.float32)
    u = np.asarray(u, dtype=np.float32)

    # --- spectral norm (one power iteration), matches reference ---
    eps = np.float32(1e-12)
    v = (W.T @ u).astype(np.float32)
    v = v / (np.float32(np.linalg.norm(v)) + eps)
    Wv = (W @ v).astype(np.float32)
    u2 = Wv / (np.float32(np.linalg.norm(Wv)) + eps)
    sigma = np.float32(u2 @ Wv)
    WnT = np.ascontiguousarray((W / sigma).T, dtype=np.float32)  # [cin, cout]
    # extended weight: col C = row-mean column (mu comes out of the matmul)
    WnT_ext = np.concatenate([WnT, WnT.mean(axis=1, keepdims=True)], axis=1)
    WnT_ext = np.ascontiguousarray(WnT_ext.astype(ml_dtypes.bfloat16))
    b_ext = np.concatenate([b, [b.mean()]]).reshape(1, CW).astype(ml_dtypes.bfloat16)
    b_nonzero = bool(np.any(b))

    src = ei[0].astype(np.int64)
    dst = ei[1].astype(np.int64)

    # --- degrees; xs = dinv * x (row-scaled source features) ---
    deg = (np.bincount(dst, minlength=N) + 1).astype(np.float32)
    dinv = (1.0 / np.sqrt(deg)).astype(np.float32)
    deg_pad = np.ones(NPAD, dtype=np.float32)
    deg_pad[:N] = deg
    sqrtdeg_pad = np.sqrt(deg_pad)

    # --- group real edges by (core, tile, src-window) ---
    # The lo window covers src [0, HALF); the hi window covers [HALF2, NPAD).
    # src in [HALF2, HALF) can go to either group: per (core, tile) we pick the
    # split so both groups pack into a shared minimal number of 128-edge blocks.
    core = dst // NPC
    tile_g = (dst % NPC) // P
    dstloc = (dst % P).astype(np.int64)
    grp = core * TPC + tile_g
    NGT = NCORES * TPC
    total_ct = np.bincount(grp, minlength=NGT).reshape(NCORES, TPC)
    nlo_fix = np.bincount(grp[src < HALF2], minlength=NGT).reshape(NCORES, TPC)
    nhi_fix = np.bincount(grp[src >= HALF], minlength=NGT).reshape(NCORES, TPC)
    nflex = total_ct - nlo_fix - nhi_fix
    B_t = np.ceil(total_ct.max(axis=0) / P).astype(np.int64)            # [TPC]
    nb_lo = np.ceil(nlo_fix.max(axis=0) / P).astype(np.int64)
    nb_hi = np.maximum(np.ceil(nhi_fix.max(axis=0) / P).astype(np.int64),
                       B_t - nb_lo)
    # flex edges assigned to lo per (core, tile)
    k_ct = np.clip(nflex + nhi_fix - nb_hi[None, :] * P, 0, nflex)
    nlo_cnt = nlo_fix + k_ct
    assert (nlo_cnt <= nb_lo[None, :] * P).all()
    assert (total_ct - nlo_cnt <= nb_hi[None, :] * P).all()
    # src-sorted rank within (core, tile): first nlo_cnt edges -> lo window
    order0 = np.lexsort((src, grp))
    starts0 = np.zeros(NGT + 1, dtype=np.int64)
    np.cumsum(np.bincount(grp, minlength=NGT), out=starts0[1:])
    rank0 = np.arange(len(grp), dtype=np.int64) - starts0[grp[order0]]
    half = np.empty(len(grp), dtype=np.int64)
    half[order0] = (rank0 >= nlo_cnt.reshape(-1)[grp[order0]]).astype(np.int64)

    key = (grp * 2 + half).astype(np.int64)
    NG = NCORES * TPC * 2
    order = np.argsort(key, kind="stable")
    counts = np.bincount(key, minlength=NG)
    starts = np.zeros(NG + 1, dtype=np.int64)
    np.cumsum(counts, out=starts[1:])
    rank = np.arange(len(key), dtype=np.int64) - starts[key[order]]

    nb = np.stack([nb_lo, nb_hi], axis=1)  # [TPC, 2] gather blocks

    # Gather-column layout per chunk: [lo blocks | hi blocks] (no self cols).
    blk_gcol = np.zeros((TPC, 2), dtype=np.int64)  # global gather column of run
    gather_gcol0 = np.zeros((NCHUNKS, 2), dtype=np.int64)
    gather_nblk = np.zeros((NCHUNKS, 2), dtype=np.int64)
    gpos = 0
    for ci in range(NCHUNKS):
        t0 = ci * CHUNK_TILES
        t1 = min(t0 + CHUNK_TILES, TPC)
        for h in range(2):
            gather_gcol0[ci, h] = gpos
            for t in range(t0, t1):
                blk_gcol[t, h] = gpos
                gpos += nb[t, h]
            gather_nblk[ci, h] = gpos - gather_gcol0[ci, h]
    totg = gpos

    # host-precomputed one-hot scatter blocks S[e, gcol, dst] (exact 0/1 fp8);
    # [e, gcol, dst]-major so a chunk's S loads as one descriptor per partition
    SBLK = np.zeros((NCORES, P, totg, P), dtype=ml_dtypes.float8_e4m3)
    IDXALL = np.zeros((NCORES, totg * P), dtype=np.int16)

    o_core = core[order]
    o_tile = tile_g[order]
    o_half = half[order]
    o_gcol = blk_gcol[o_tile, o_half] + rank // P
    o_slot = rank % P

    SBLK[o_core, o_slot, o_gcol, dstloc[order]] = 1.0
    IDXALL[o_core, o_gcol * P + o_slot] = (src[order] - o_half * HALF2).astype(np.int16)

    # idx SBUF layout: pos k -> [k % 16, k // 16], replicated 8x over partitions
    IDX = np.tile(IDXALL.reshape(NCORES, -1, 16).transpose(0, 2, 1), (1, 8, 1))
    n0 = int(gather_nblk[0].sum())  # chunk-0 gather blocks (loaded first)

    xs_pad = np.zeros((NPAD, C), dtype=ml_dtypes.bfloat16)
    xs_pad[:N] = (dinv[:, None] * x).astype(ml_dtypes.bfloat16)
    # feature-major self rows: XST[core][c, d_local] (bf16)
    XST = np.ascontiguousarray(
        xs_pad.reshape(NCORES, NPC, C).transpose(0, 2, 1)
    )

    IDENT = np.eye(P, dtype=ml_dtypes.bfloat16)

    SQRTDEG = sqrtdeg_pad.reshape(NCORES, 1, NPC).astype(ml_dtypes.bfloat16)
    EPSDEG = (EPS_IN * deg_pad).reshape(NCORES, TPC, P).transpose(0, 2, 1)
    EPSDEG = np.ascontiguousarray(EPSDEG, dtype=np.float32)  # [NCORES, P, TPC]

    meta = dict(
        nb=nb,
        blk_gcol=blk_gcol,
        gather_gcol0=gather_gcol0,
        gather_nblk=gather_nblk,
        totg=totg,
        n0=n0,
        b_nonzero=b_nonzero,
    )
    global _N0
    _N0 = n0
    return xs_pad, XST, IDX, SBLK, IDENT, SQRTDEG, EPSDEG, WnT_ext, b_ext, meta


def _build(meta):
    """Build the SPMD Bass graph (shared across all 8 cores)."""
    nb = meta["nb"]
    b_nonzero = meta["b_nonzero"]
    blk_gcol = meta["blk_gcol"]
    gather_gcol0 = meta["gather_gcol0"]
    gather_nblk = meta["gather_nblk"]
    totg = meta["totg"]

    nc = bacc.Bacc(
        "TRN2", target_bir_lowering=False, debug=False, num_swdge_queues=NQ
    )

    x_d = nc.dram_tensor("x", [NPAD, C], mybir.dt.bfloat16, kind="ExternalInput")
    xst_d = nc.dram_tensor("xst", [C, NPC], mybir.dt.bfloat16, kind="ExternalInput")
    n0 = meta["n0"]
    idx0_d = nc.dram_tensor("idx0", [P, n0 * 8], mybir.dt.int16, kind="ExternalInput")
    idx_d = nc.dram_tensor("idx", [P, totg * 8], mybir.dt.int16, kind="ExternalInput")
    s_d = nc.dram_tensor("s", [P, totg * P], mybir.dt.float8e4, kind="ExternalInput")
    ident_d = nc.dram_tensor("ident", [P, P], mybir.dt.bfloat16, kind="ExternalInput")
    sd_d = nc.dram_tensor("sqrtdeg", [1, NPC], mybir.dt.bfloat16, kind="ExternalInput")
    epsdeg_d = nc.dram_tensor("epsdeg", [P, TPC], mybir.dt.float32, kind="ExternalInput")
    wnT_d = nc.dram_tensor("wnT", [C, CW], mybir.dt.bfloat16, kind="ExternalInput")
    b_d = nc.dram_tensor("b", [1, CW], mybir.dt.bfloat16, kind="ExternalInput")
    out_d = nc.dram_tensor("out", [NPC, C], mybir.dt.bfloat16, kind="ExternalOutput")

    # max gather blocks per chunk for each half (separate tiles per half)
    nlo_max = max(int(gather_nblk[ci, 0]) for ci in range(NCHUNKS))
    nhi_max = max(int(gather_nblk[ci, 1]) for ci in range(NCHUNKS))
    nblk_max = max(int(gather_nblk[ci].sum()) for ci in range(NCHUNKS))

    qctr = 0  # gather round-robin queue counter

    with tile.TileContext(nc) as tc, ExitStack() as ctx:
        meta_p = ctx.enter_context(tc.tile_pool(name="meta", bufs=1))
        gat_p = ctx.enter_context(tc.tile_pool(name="gat", bufs=7))
        s_p = ctx.enter_context(tc.tile_pool(name="s", bufs=6))
        agg_p = ctx.enter_context(tc.tile_pool(name="agg", bufs=4))
        out_p = ctx.enter_context(tc.tile_pool(name="out", bufs=6))
        col_p = ctx.enter_context(tc.tile_pool(name="col", bufs=16))
        small_p = ctx.enter_context(tc.tile_pool(name="small", bufs=1))
        ps_p = ctx.enter_context(tc.tile_pool(name="ps", bufs=8, space="PSUM"))

        idxz = meta_p.tile([P, 8], mybir.dt.int16)
        nc.vector.memset(idxz[:], 0)
        warm = meta_p.tile([P, 1, P], mybir.dt.bfloat16)
        nc.gpsimd.dma_gather(
            out_ap=warm[:], in_ap=x_d[0:HALF, :], idxs_ap=idxz[:, 0:8],
            num_idxs=P, num_idxs_reg=P, elem_size=C,
            single_packet=False, queue_num=0,
        )
        idx0_sb = meta_p.tile([P, n0 * 8], mybir.dt.int16)
        nc.sync.dma_start(idx0_sb[:], idx0_d[:])
        idx_sb = meta_p.tile([P, totg * 8], mybir.dt.int16)
        nc.sync.dma_start(idx_sb[:], idx_d[:])
        ident_sb = meta_p.tile([P, P], mybir.dt.bfloat16)
        nc.sync.dma_start(ident_sb[:], ident_d[:])
        xst_sb = meta_p.tile([C, NPC], mybir.dt.bfloat16)
        nc.sync.dma_start(xst_sb[:], xst_d[:])
        epsdeg_sb = meta_p.tile([P, TPC], mybir.dt.float32)
        nc.sync.dma_start(epsdeg_sb[:], epsdeg_d[:])
        wnT_sb = meta_p.tile([C, CW], mybir.dt.bfloat16)
        nc.sync.dma_start(wnT_sb[:], wnT_d[:])
        if b_nonzero:
            sd_sb = meta_p.tile([1, NPC], mybir.dt.bfloat16)
            nc.sync.dma_start(sd_sb[:], sd_d[:])
            b_sb = meta_p.tile([1, CW], mybir.dt.bfloat16)
            nc.sync.dma_start(b_sb[:], b_d[:])

        x_lo = x_d[0:HALF, :]
        x_hi = x_d[HALF2 : HALF2 + HALF, :]

        for ci in range(NCHUNKS):
            t0 = ci * CHUNK_TILES
            t1 = min(t0 + CHUNK_TILES, TPC)
            g0 = int(gather_gcol0[ci, 0])
            nblk_ci = int(gather_nblk[ci].sum())

            s_sb = s_p.tile([P, nblk_max * P], mybir.dt.float8e4, tag="sblk")
            nc.sync.dma_start(
                s_sb[:, 0 : nblk_ci * P], s_d[:, g0 * P : (g0 + nblk_ci) * P]
            )
            gat_lo = gat_p.tile([P, nlo_max, P], mybir.dt.bfloat16, tag="glo")
            gat_hi = gat_p.tile([P, nhi_max, P], mybir.dt.bfloat16, tag="ghi")
            gat_half = [gat_lo, gat_hi]

            # gathers: separate dst tiles per half -> no WAW between them, so
            # up to 4 gathers (2 chunks x 2 halves) run on 4 Q7 pairs at once.
            # Alternate half order per chunk so queue round-robin spreads the
            # (larger) lo and (smaller) hi calls evenly.
            halves = ((0, x_lo), (1, x_hi)) if ci % 2 == 0 else ((1, x_hi), (0, x_lo))
            for h, src_ap in halves:
                nblk_g = int(gather_nblk[ci, h])
                if nblk_g == 0:
                    continue
                # split into two sub-gathers on different queues; rotate the
                # queue offset per chunk so lo/hi sizes balance across queues
                nb1 = (nblk_g + 1) // 2
                for b0, b1 in ((0, nb1), (nb1, nblk_g)):
                    if b1 <= b0:
                        continue
                    nidx = (b1 - b0) * P
                    ic0 = (int(gather_gcol0[ci, h]) + b0) * 8
                    idx_src = idx0_sb if ci == 0 else idx_sb
                    nc.gpsimd.dma_gather(
                        out_ap=gat_half[h][:, b0:b1, :],
                        in_ap=src_ap,
                        idxs_ap=idx_src[:, ic0 : ic0 + nidx // 16],
                        num_idxs=nidx,
                        num_idxs_reg=nidx,
                        elem_size=C,
                        single_packet=False,
                        queue_num=(qctr + ci) % NQ,
                    )
                    qctr += 1

            for t in range(t0, t1):
                # (source tile, source column, S column) per gather block
                blocks = []
                for h in range(2):
                    loc0 = int(blk_gcol[t, h]) - int(gather_gcol0[ci, h])
                    scol0 = int(blk_gcol[t, h]) - g0
                    for j in range(int(nb[t, h])):
                        blocks.append((gat_half[h], loc0 + j, scol0 + j))

                pt = ps_p.tile([P, C], mybir.dt.float32, tag="ps")
                # self rows first: aggT += I.T @ xsT_slice
                nc.tensor.matmul(
                    pt[:], lhsT=ident_sb[:], rhs=xst_sb[:, t * P : (t + 1) * P],
                    start=True, stop=False,
                )
                for j, (gtile, gcol, scol) in enumerate(blocks):
                    nc.tensor.matmul(
                        pt[:],
                        lhsT=gtile[:, gcol, :],
                        rhs=s_sb[:, scol * P : (scol + 1) * P],
                        start=False,
                        stop=(j == len(blocks) - 1),
                    )

                aggt = agg_p.tile([P, C], mybir.dt.bfloat16, tag="aggt")
                nc.scalar.copy(aggt[:], pt[:])

                # ---- weight matmul + per-tile InstanceNorm, all ACT ----
                po = ps_p.tile([P, CW], mybir.dt.float32, tag="ps")
                nc.tensor.matmul(
                    po[:], lhsT=aggt[:], rhs=wnT_sb[:],
                    start=True, stop=not b_nonzero,
                )
                if b_nonzero:
                    # bias' = sqrt(deg) * b (per-dst row scale folded into lhsT)
                    nc.tensor.matmul(
                        po[:], lhsT=sd_sb[:, t * P : (t + 1) * P], rhs=b_sb[:],
                        start=False, stop=True,
                    )
                sqj = small_p.tile([P, P], mybir.dt.bfloat16, tag="sqj")
                ssqc = col_p.tile([P, 1], mybir.dt.float32, tag="ssq")
                nc.scalar.activation(
                    out=sqj[:], in_=po[:, 0:C],
                    func=mybir.ActivationFunctionType.Square,
                    accum_out=ssqc[:],
                )
                negmu = col_p.tile([P, 1], mybir.dt.float32, tag="negmu")
                nc.scalar.activation(
                    out=negmu[:], in_=po[:, C : C + 1],
                    func=mybir.ActivationFunctionType.Identity, scale=-1.0,
                )
                m2 = col_p.tile([P, 1], mybir.dt.float32, tag="m2")
                nc.scalar.activation(
                    out=m2[:], in_=negmu[:],
                    func=mybir.ActivationFunctionType.Square,
                )
                # t2 = ssq/C + deg*eps
                t2 = col_p.tile([P, 1], mybir.dt.float32, tag="t2")
                nc.scalar.activation(
                    out=t2[:], in_=ssqc[:],
                    func=mybir.ActivationFunctionType.Identity,
                    scale=1.0 / C, bias=epsdeg_sb[:, t : t + 1],
                )
                # rstd = rsqrt(t2 - mu^2) via Exp(-0.5*Ln(.)) (Rsqrt is
                # blocked for accuracy; Ln/Exp measured exact to ~2e-7)
                lnv = col_p.tile([P, 1], mybir.dt.float32, tag="lnv")
                nc.scalar.activation(
                    out=lnv[:], in_=m2[:],
                    func=mybir.ActivationFunctionType.Ln,
                    scale=-1.0, bias=t2[:],
                )
                rstd = col_p.tile([P, 1], mybir.dt.float32, tag="rstd")
                nc.scalar.activation(
                    out=rstd[:], in_=lnv[:],
                    func=mybir.ActivationFunctionType.Exp, scale=-0.5,
                )
                # nmr = -mu * rstd
                nmr = col_p.tile([P, 1], mybir.dt.float32, tag="nmr")
                nc.scalar.activation(
                    out=nmr[:], in_=negmu[:],
                    func=mybir.ActivationFunctionType.Identity, scale=rstd[:],
                )
                final = out_p.tile([P, P], mybir.dt.bfloat16, tag="final")
                # fused normalize + LeakyReLU: Prelu(po*rstd - mu*rstd, alpha=0.2)
                nc.scalar.activation(
                    out=final[:], in_=po[:, 0:C],
                    func=mybir.ActivationFunctionType.Prelu,
                    bias=nmr[:], scale=rstd[:], alpha=0.2,
                )
                nc.sync.dma_start(out_d[t * P : (t + 1) * P, :], final[:])

    nc.compile()
    return nc


_N0 = None


def _make_in_maps(xs_pad, XST, IDX, SBLK, IDENT, SQRTDEG, EPSDEG, WnT_ext, b_ext):
    return [
        {
            "x": xs_pad,
            "xst": np.ascontiguousarray(XST[i]),
            "idx": np.ascontiguousarray(IDX[i]),
            "idx0": np.ascontiguousarray(IDX[i][:, : _N0 * 8]),
            "s": np.ascontiguousarray(SBLK[i].reshape(P, -1)),
            "ident": IDENT,
            "sqrtdeg": np.ascontiguousarray(SQRTDEG[i]),
            "epsdeg": np.ascontiguousarray(EPSDEG[i]),
            "wnT": WnT_ext,
            "b": b_ext,
        }
        for i in range(NCORES)
    ]


def kernel(x, edge_index, W, b, u):
    pre = _preprocess(x, edge_index, W, b, u)
    nc = _build(pre[-1])
    in_maps = _make_in_maps(*pre[:-1])

    # The axon terminal can be transiently unavailable right after a prior
    # process's teardown; retry with backoff.
    import time

    last_err = None
    for attempt in range(6):
        try:
            res = run_bass_kernel_spmd(nc, in_maps, list(range(NCORES)))
            break
        except Exception as e:  # noqa: BLE001
            last_err = e
            time.sleep(45)
    else:
        raise last_err
    shards = [np.asarray(res.results[i]["out"]) for i in range(NCORES)]
    out = np.concatenate(shards, axis=0)[:N]
    return out.astype(np.float32)


# revision 14
# speedup vs baseline: 1.5955x; 1.0967x over previous
"""Trainium2 Bass kernel for GCNBlock (spectral-norm linear + GCN aggregation +
InstanceNorm + LeakyReLU) distributed across 8 NeuronCores.

v3 — single-pass, DVE-free hot loop.

Strategy (dst-sharded):
  - out = (A @ xs) @ WnT with xs = dinv*x host-prescaled; per-dst dinv folded
    into eps' = deg*eps (and bias' = sqrt(deg)*b when b != 0), so scatter
    matrices are pure one-hot (host-precomputed fp8, streamed per chunk at
    line rate).
  - dst nodes sharded across 8 cores (49 tiles of 128 per core); edges
    partitioned by (tile, src-half) into 128-edge blocks; per block a SWDGE
    dma_gather pulls the 128 source rows (bf16) and the PE accumulates
    aggT[cin, dst] += Xsrc.T @ S in PSUM across 4 SWDGE queues.

Learned constraints driving this shape:
  - DVE is OFF LIMITS in the hot loop: the Q7 SWDGE desc-gen shares DVE's
    SBUF port pair, and any in-loop DVE op serializes against the gathers
    (measured: [128,7] DVE ops at 8.5us during desc-gen; gathers slow too).
    All elementwise work is therefore ACT-only, restructured as unary
    activation(func, scale, bias) chains with per-partition [P,1] APs.
  - 16 SDMA engines' per-packet dwell is the roofline: v1 spent 66us/engine
    on HWDGE packets.  v3 cuts HWDGE bytes: self rows come from one resident
    feature-major xsT load (128 big descs, folded into the PSUM chain as
    matmul(lhsT=I, rhs=xsT_slice)) instead of 6.3k small loads; output is
    bf16 (host upcasts); weights/aux are bf16.
  - The tail is gone: everything streams per tile inside the hot loop.
    After a tile's aggregation chain drains (ACT copy pt->aggT scratch), one
    PE matmul forms po = aggT.T @ WnT_ext (129th col = row mean) in PSUM,
    then an ACT chain computes Square+accum -> ssq, negmu, mu^2, t2 =
    ssq/C + deg*eps, rstd = Rsqrt(-mu^2 + t2), nmr = negmu*rstd, and the
    fused Prelu(po*rstd + nmr) -> bf16 out, with po read directly from PSUM
    (no second weight matmul, no persistent agg_all).  The PE stays
    continuously busy so HAM reaches full rate.
"""

import numpy as np
import ml_dtypes
from contextlib import ExitStack

import concourse.tile as tile
from concourse import bacc, mybir
from concourse.bass_utils import run_bass_kernel_spmd

_N0 = None

# Problem constants (hardcoded per spec)
N, E, C = 50000, 800000, 128
P = 128
NCORES = 8
TPC = 49                # dst tiles per core
NPC = TPC * P           # 6272 dst nodes per core
NPAD = NCORES * NPC     # 50176 padded node count
HALF = 32768            # int16 index window size
HALF2 = NPAD - HALF     # hi-window base (17408); [HALF2, HALF) rows are flexible
CHUNK_TILES = 3
NCHUNKS = -(-TPC // CHUNK_TILES)  # 17 (last chunk ragged)
NQ = 4                  # SWDGE queues (Q7 core pairs) used for gathers
EPS_IN = 1e-5
CW = C + 1              # weight matmul width (extra column = row mean)
BAT = 17                # tiles per batched-rsqrt group (bounds ACT table loads)


def _preprocess(x, edge_index, W, b, u):
    """Host-side prep: spectral norm, edge partitioning, metadata layout."""
    x = np.asarray(x, dtype=np.float32)
    ei = np.asarray(edge_index)
    W = np.asarray(W, dtype=np.float32)
    b = np.asarray(b, dtype=np.float32)
    u = np.asarray(u, dtype=np.float32)

    # --- spectral norm (one power iteration), matches reference ---
    eps = np.float32(1e-12)
    v = (W.T @ u).astype(np.float32)
    v = v / (np.float32(np.linalg.norm(v)) + eps)
    Wv = (W @ v).astype(np.float32)
    u2 = Wv / (np.float32(np.linalg.norm(Wv)) + eps)
    sigma = np.float32(u2 @ Wv)
    WnT = np.ascontiguousarray((W / sigma).T, dtype=np.float32)  # [cin, cout]
    # extended weight: col C = row-mean column (mu comes out of the matmul)
    WnT_ext = np.concatenate([WnT, WnT.mean(axis=1, keepdims=True)], axis=1)
    WnT_ext = np.ascontiguousarray(WnT_ext.astype(ml_dtypes.bfloat16))
    b_ext = np.concatenate([b, [b.mean()]]).reshape(1, CW).astype(ml_dtypes.bfloat16)
    b_nonzero = bool(np.any(b))

    src = ei[0].astype(np.int64)
    dst = ei[1].astype(np.int64)

    # --- degrees; xs = dinv * x (row-scaled source features) ---
    deg = (np.bincount(dst, minlength=N) + 1).astype(np.float32)
    dinv = (1.0 / np.sqrt(deg)).astype(np.float32)
    deg_pad = np.ones(NPAD, dtype=np.float32)
    deg_pad[:N] = deg
    sqrtdeg_pad = np.sqrt(deg_pad)

    # --- group real edges by (core, tile, src-window) ---
    # The lo window covers src [0, HALF); the hi window covers [HALF2, NPAD).
    # src in [HALF2, HALF) can go to either group: per (core, tile) we pick the
    # split so both groups pack into a shared minimal number of 128-edge blocks.
    core = dst // NPC
    tile_g = (dst % NPC) // P
    dstloc = (dst % P).astype(np.int64)
    grp = core * TPC + tile_g
    NGT = NCORES * TPC
    total_ct = np.bincount(grp, minlength=NGT).reshape(NCORES, TPC)
    nlo_fix = np.bincount(grp[src < HALF2], minlength=NGT).reshape(NCORES, TPC)
    nhi_fix = np.bincount(grp[src >= HALF], minlength=NGT).reshape(NCORES, TPC)
    nflex = total_ct - nlo_fix - nhi_fix
    B_t = np.ceil(total_ct.max(axis=0) / P).astype(np.int64)            # [TPC]
    nb_lo = np.ceil(nlo_fix.max(axis=0) / P).astype(np.int64)
    nb_hi = np.maximum(np.ceil(nhi_fix.max(axis=0) / P).astype(np.int64),
                       B_t - nb_lo)
    # flex edges assigned to lo per (core, tile)
    k_ct = np.clip(nflex + nhi_fix - nb_hi[None, :] * P, 0, nflex)
    nlo_cnt = nlo_fix + k_ct
    assert (nlo_cnt <= nb_lo[None, :] * P).all()
    assert (total_ct - nlo_cnt <= nb_hi[None, :] * P).all()
    # src-sorted rank within (core, tile): first nlo_cnt edges -> lo window
    order0 = np.lexsort((src, grp))
    starts0 = np.zeros(NGT + 1, dtype=np.int64)
    np.cumsum(np.bincount(grp, minlength=NGT), out=starts0[1:])
    rank0 = np.arange(len(grp), dtype=np.int64) - starts0[grp[order0]]
    half = np.empty(len(grp), dtype=np.int64)
    half[order0] = (rank0 >= nlo_cnt.reshape(-1)[grp[order0]]).astype(np.int64)

    key = (grp * 2 + half).astype(np.int64)
    NG = NCORES * TPC * 2
    order = np.argsort(key, kind="stable")
    counts = np.bincount(key, minlength=NG)
    starts = np.zeros(NG + 1, dtype=np.int64)
    np.cumsum(counts, out=starts[1:])
    rank = np.arange(len(key), dtype=np.int64) - starts[key[order]]

    nb = np.stack([nb_lo, nb_hi], axis=1)  # [TPC, 2] gather blocks

    # Gather-column layout per chunk: [lo blocks | hi blocks] (no self cols).
    blk_gcol = np.zeros((TPC, 2), dtype=np.int64)  # global gather column of run
    gather_gcol0 = np.zeros((NCHUNKS, 2), dtype=np.int64)
    gather_nblk = np.zeros((NCHUNKS, 2), dtype=np.int64)
    gpos = 0
    for ci in range(NCHUNKS):
        t0 = ci * CHUNK_TILES
        t1 = min(t0 + CHUNK_TILES, TPC)
        for h in range(2):
            gather_gcol0[ci, h] = gpos
            for t in range(t0, t1):
                blk_gcol[t, h] = gpos
                gpos += nb[t, h]
            gather_nblk[ci, h] = gpos - gather_gcol0[ci, h]
    totg = gpos

    # host-precomputed one-hot scatter blocks S[e, gcol, dst] (exact 0/1 fp8);
    # [e, gcol, dst]-major so a chunk's S loads as one descriptor per partition
    SBLK = np.zeros((NCORES, P, totg, P), dtype=ml_dtypes.float8_e4m3)
    IDXALL = np.zeros((NCORES, totg * P), dtype=np.int16)

    o_core = core[order]
    o_tile = tile_g[order]
    o_half = half[order]
    o_gcol = blk_gcol[o_tile, o_half] + rank // P
    o_slot = rank % P

    SBLK[o_core, o_slot, o_gcol, dstloc[order]] = 1.0
    IDXALL[o_core, o_gcol * P + o_slot] = (src[order] - o_half * HALF2).astype(np.int16)

    # idx SBUF layout: pos k -> [k % 16, k // 16], replicated 8x over partitions
    IDX = np.tile(IDXALL.reshape(NCORES, -1, 16).transpose(0, 2, 1), (1, 8, 1))
    n0 = int(gather_nblk[0].sum())  # chunk-0 gather blocks (loaded first)

    xs_pad = np.zeros((NPAD, C), dtype=ml_dtypes.bfloat16)
    xs_pad[:N] = (dinv[:, None] * x).astype(ml_dtypes.bfloat16)
    # feature-major self rows: XST[core][c, d_local] (bf16)
    XST = np.ascontiguousarray(
        xs_pad.reshape(NCORES, NPC, C).transpose(0, 2, 1)
    )

    IDENT = np.eye(P, dtype=ml_dtypes.bfloat16)

    SQRTDEG = sqrtdeg_pad.reshape(NCORES, 1, NPC).astype(ml_dtypes.bfloat16)
    EPSDEG = (EPS_IN * deg_pad).reshape(NCORES, TPC, P).transpose(0, 2, 1)
    EPSDEG = np.ascontiguousarray(EPSDEG, dtype=np.float32)  # [NCORES, P, TPC]

    meta = dict(
        nb=nb,
        blk_gcol=blk_gcol,
        gather_gcol0=gather_gcol0,
        gather_nblk=gather_nblk,
        totg=totg,
        n0=n0,
        b_nonzero=b_nonzero,
    )
    global _N0
    _N0 = n0
    return xs_pad, XST, IDX, SBLK, IDENT, SQRTDEG, EPSDEG, WnT_ext, b_ext, meta


def _build(meta):
    """Build the SPMD Bass graph (shared across all 8 cores)."""
    nb = meta["nb"]
    b_nonzero = meta["b_nonzero"]
    blk_gcol = meta["blk_gcol"]
    gather_gcol0 = meta["gather_gcol0"]
    gather_nblk = meta["gather_nblk"]
    totg = meta["totg"]

    nc = bacc.Bacc(
        "TRN2", target_bir_lowering=False, debug=False, num_swdge_queues=NQ
    )

    x_d = nc.dram_tensor("x", [NPAD, C], mybir.dt.bfloat16, kind="ExternalInput")
    xst_d = nc.dram_tensor("xst", [C, NPC], mybir.dt.bfloat16, kind="ExternalInput")
    n0 = meta["n0"]
    idx0_d = nc.dram_tensor("idx0", [P, n0 * 8], mybir.dt.int16, kind="ExternalInput")
    idx_d = nc.dram_tensor("idx", [P, totg * 8], mybir.dt.int16, kind="ExternalInput")
    s_d = nc.dram_tensor("s", [P, totg * P], mybir.dt.float8e4, kind="ExternalInput")
    ident_d = nc.dram_tensor("ident", [P, P], mybir.dt.bfloat16, kind="ExternalInput")
    sd_d = nc.dram_tensor("sqrtdeg", [1, NPC], mybir.dt.bfloat16, kind="ExternalInput")
    epsdeg_d = nc.dram_tensor("epsdeg", [P, TPC], mybir.dt.float32, kind="ExternalInput")
    wnT_d = nc.dram_tensor("wnT", [C, CW], mybir.dt.bfloat16, kind="ExternalInput")
    b_d = nc.dram_tensor("b", [1, CW], mybir.dt.bfloat16, kind="ExternalInput")
    out_d = nc.dram_tensor("out", [NPC, C], mybir.dt.bfloat16, kind="ExternalOutput")

    # max gather blocks per chunk for each half (separate tiles per half)
    nlo_max = max(int(gather_nblk[ci, 0]) for ci in range(NCHUNKS))
    nhi_max = max(int(gather_nblk[ci, 1]) for ci in range(NCHUNKS))
    nblk_max = max(int(gather_nblk[ci].sum()) for ci in range(NCHUNKS))

    qctr = 1  # gather round-robin queue counter (q0 starts busy with warm-up)

    with tile.TileContext(nc) as tc, ExitStack() as ctx:
        meta_p = ctx.enter_context(tc.tile_pool(name="meta", bufs=1))
        gat_p = ctx.enter_context(tc.tile_pool(name="gat", bufs=7))
        s_p = ctx.enter_context(tc.tile_pool(name="s", bufs=6))
        agg_p = ctx.enter_context(tc.tile_pool(name="agg", bufs=4))
        out_p = ctx.enter_context(tc.tile_pool(name="out", bufs=BAT + 3))
        col_p = ctx.enter_context(tc.tile_pool(name="col", bufs=16))
        small_p = ctx.enter_context(tc.tile_pool(name="small", bufs=1))
        ps_p = ctx.enter_context(tc.tile_pool(name="ps", bufs=8, space="PSUM"))

        idxz = meta_p.tile([P, 8], mybir.dt.int16)
        nc.vector.memset(idxz[:], 0)
        warm = meta_p.tile([P, 1, P], mybir.dt.bfloat16)
        nc.gpsimd.dma_gather(
            out_ap=warm[:], in_ap=x_d[0:HALF, :], idxs_ap=idxz[:, 0:8],
            num_idxs=P, num_idxs_reg=P, elem_size=C,
            single_packet=False, queue_num=0,
        )
        idx0_sb = meta_p.tile([P, n0 * 8], mybir.dt.int16)
        nc.sync.dma_start(idx0_sb[:], idx0_d[:])
        idx_sb = meta_p.tile([P, totg * 8], mybir.dt.int16)
        nc.sync.dma_start(idx_sb[:], idx_d[:])
        ident_sb = meta_p.tile([P, P], mybir.dt.bfloat16)
        nc.sync.dma_start(ident_sb[:], ident_d[:])
        xst_sb = meta_p.tile([C, NPC], mybir.dt.bfloat16)
        nc.sync.dma_start(xst_sb[:], xst_d[:])
        epsdeg_sb = meta_p.tile([P, TPC], mybir.dt.float32)
        nc.sync.dma_start(epsdeg_sb[:], epsdeg_d[:])
        wnT_sb = meta_p.tile([C, CW], mybir.dt.bfloat16)
        nc.sync.dma_start(wnT_sb[:], wnT_d[:])
        if b_nonzero:
            sd_sb = meta_p.tile([1, NPC], mybir.dt.bfloat16)
            nc.sync.dma_start(sd_sb[:], sd_d[:])
            b_sb = meta_p.tile([1, CW], mybir.dt.bfloat16)
            nc.sync.dma_start(b_sb[:], b_d[:])

        x_lo = x_d[0:HALF, :]
        x_hi = x_d[HALF2 : HALF2 + HALF, :]

        # staging for the batched rsqrt + deferred rstd scaling
        t2_all = meta_p.tile([P, TPC], mybir.dt.float32)
        rstd_all = meta_p.tile([P, TPC], mybir.dt.float32)
        fin0_tiles = {}

        for ci in range(NCHUNKS):
            t0 = ci * CHUNK_TILES
            t1 = min(t0 + CHUNK_TILES, TPC)
            g0 = int(gather_gcol0[ci, 0])
            nblk_ci = int(gather_nblk[ci].sum())

            s_sb = s_p.tile([P, nblk_max * P], mybir.dt.float8e4, tag="sblk")
            nc.sync.dma_start(
                s_sb[:, 0 : nblk_ci * P], s_d[:, g0 * P : (g0 + nblk_ci) * P]
            )
            gat_lo = gat_p.tile([P, nlo_max, P], mybir.dt.bfloat16, tag="glo")
            gat_hi = gat_p.tile([P, nhi_max, P], mybir.dt.bfloat16, tag="ghi")
            gat_half = [gat_lo, gat_hi]

            # gathers: separate dst tiles per half -> no WAW between them, so
            # up to 4 gathers (2 chunks x 2 halves) run on 4 Q7 pairs at once.
            # Alternate half order per chunk so queue round-robin spreads the
            # (larger) lo and (smaller) hi calls evenly.
            halves = ((0, x_lo), (1, x_hi)) if ci % 2 == 0 else ((1, x_hi), (0, x_lo))
            for h, src_ap in halves:
                nblk_g = int(gather_nblk[ci, h])
                if nblk_g == 0:
                    continue
                # split into two sub-gathers on different queues; rotate the
                # queue offset per chunk so lo/hi sizes balance across queues
                nb1 = (nblk_g + 1) // 2
                for b0, b1 in ((0, nb1), (nb1, nblk_g)):
                    if b1 <= b0:
                        continue
                    nidx = (b1 - b0) * P
                    ic0 = (int(gather_gcol0[ci, h]) + b0) * 8
                    idx_src = idx0_sb if ci == 0 else idx_sb
                    nc.gpsimd.dma_gather(
                        out_ap=gat_half[h][:, b0:b1, :],
                        in_ap=src_ap,
                        idxs_ap=idx_src[:, ic0 : ic0 + nidx // 16],
                        num_idxs=nidx,
                        num_idxs_reg=nidx,
                        elem_size=C,
                        single_packet=False,
                        queue_num=(qctr + ci) % NQ,
                    )
                    qctr += 1

            for t in range(t0, t1):
                # (source tile, source column, S column) per gather block
                blocks = []
                for h in range(2):
                    loc0 = int(blk_gcol[t, h]) - int(gather_gcol0[ci, h])
                    scol0 = int(blk_gcol[t, h]) - g0
                    for j in range(int(nb[t, h])):
                        blocks.append((gat_half[h], loc0 + j, scol0 + j))

                pt = ps_p.tile([P, C], mybir.dt.float32, tag="ps")
                # self rows first: aggT += I.T @ xsT_slice
                nc.tensor.matmul(
                    pt[:], lhsT=ident_sb[:], rhs=xst_sb[:, t * P : (t + 1) * P],
                    start=True, stop=False,
                )
                for j, (gtile, gcol, scol) in enumerate(blocks):
                    nc.tensor.matmul(
                        pt[:],
                        lhsT=gtile[:, gcol, :],
                        rhs=s_sb[:, scol * P : (scol + 1) * P],
                        start=False,
                        stop=(j == len(blocks) - 1),
                    )

                aggt = agg_p.tile([P, C], mybir.dt.bfloat16, tag="aggt")
                nc.scalar.copy(aggt[:], pt[:])

                # ---- weight matmul + per-tile stats; only table-free ACT
                # funcs (Identity/Square/Prelu) run per tile.  LeakyReLU
                # commutes with the positive rstd scale, so we emit
                # fin0 = Prelu(h - mu) now and multiply by rstd (computed by
                # a batched Ln/Exp rsqrt every BAT tiles) afterwards. ----
                po = ps_p.tile([P, CW], mybir.dt.float32, tag="ps")
                nc.tensor.matmul(
                    po[:], lhsT=aggt[:], rhs=wnT_sb[:],
                    start=True, stop=not b_nonzero,
                )
                if b_nonzero:
                    # bias' = sqrt(deg) * b (per-dst row scale folded into lhsT)
                    nc.tensor.matmul(
                        po[:], lhsT=sd_sb[:, t * P : (t + 1) * P], rhs=b_sb[:],
                        start=False, stop=True,
                    )
                negmu = col_p.tile([P, 1], mybir.dt.float32, tag="negmu")
                nc.scalar.activation(
                    out=negmu[:], in_=po[:, C : C + 1],
                    func=mybir.ActivationFunctionType.Identity, scale=-1.0,
                )
                # centered sum of squares: ssq = sum_j (h_j - mu)^2 = C*var
                sqj = small_p.tile([P, P], mybir.dt.bfloat16, tag="sqj")
                ssqc = col_p.tile([P, 1], mybir.dt.float32, tag="ssq")
                nc.scalar.activation(
                    out=sqj[:], in_=po[:, 0:C],
                    func=mybir.ActivationFunctionType.Square,
                    bias=negmu[:], accum_out=ssqc[:],
                )
                # t2 = var + deg*eps  (column t of the batch-Ln staging tile)
                nc.scalar.activation(
                    out=t2_all[:, t : t + 1], in_=ssqc[:],
                    func=mybir.ActivationFunctionType.Identity,
                    scale=1.0 / C, bias=epsdeg_sb[:, t : t + 1],
                )
                # fin0 = LeakyReLU(h - mu); final scale by rstd deferred
                fin0 = out_p.tile([P, P], mybir.dt.bfloat16, tag="fin0")
                nc.scalar.activation(
                    out=fin0[:], in_=po[:, 0:C],
                    func=mybir.ActivationFunctionType.Prelu,
                    bias=negmu[:], alpha=0.2,
                )
                fin0_tiles[t] = fin0

                if t + 1 == TPC or (t + 1) % BAT == 0:
                    # batched rsqrt: rstd = Exp(-0.5*Ln(var + eps'))
                    tb0 = (t // BAT) * BAT
                    nbt = t + 1 - tb0
                    lnv = small_p.tile([P, BAT], mybir.dt.float32, tag=f"lnv{t//BAT}")
                    nc.scalar.activation(
                        out=lnv[:, 0:nbt], in_=t2_all[:, tb0 : t + 1],
                        func=mybir.ActivationFunctionType.Ln,
                    )
                    nc.scalar.activation(
                        out=rstd_all[:, tb0 : t + 1], in_=lnv[:, 0:nbt],
                        func=mybir.ActivationFunctionType.Exp, scale=-0.5,
                    )
                    for tt in range(tb0, t + 1):
                        final = out_p.tile([P, P], mybir.dt.bfloat16, tag="final")
                        nc.scalar.activation(
                            out=final[:], in_=fin0_tiles[tt][:],
                            func=mybir.ActivationFunctionType.Identity,
                            scale=rstd_all[:, tt : tt + 1],
                        )
                        nc.sync.dma_start(out_d[tt * P : (tt + 1) * P, :], final[:])

    nc.compile()
    return nc


_N0 = None


def _make_in_maps(xs_pad, XST, IDX, SBLK, IDENT, SQRTDEG, EPSDEG, WnT_ext, b_ext):
    return [
        {
            "x": xs_pad,
            "xst": np.ascontiguousarray(XST[i]),
            "idx": np.ascontiguousarray(IDX[i]),
            "idx0": np.ascontiguousarray(IDX[i][:, : _N0 * 8]),
            "s": np.ascontiguousarray(SBLK[i].reshape(P, -1)),
            "ident": IDENT,
            "sqrtdeg": np.ascontiguousarray(SQRTDEG[i]),
            "epsdeg": np.ascontiguousarray(EPSDEG[i]),
            "wnT": WnT_ext,
            "b": b_ext,
        }
        for i in range(NCORES)
    ]


def kernel(x, edge_index, W, b, u):
    pre = _preprocess(x, edge_index, W, b, u)
    nc = _build(pre[-1])
    in_maps = _make_in_maps(*pre[:-1])

    # The axon terminal can be transiently unavailable right after a prior
    # process's teardown; retry with backoff.
    import time

    last_err = None
    for attempt in range(6):
        try:
            res = run_bass_kernel_spmd(nc, in_maps, list(range(NCORES)))
            break
        except Exception as e:  # noqa: BLE001
            last_err = e
            time.sleep(45)
    else:
        raise last_err
    shards = [np.asarray(res.results[i]["out"]) for i in range(NCORES)]
    out = np.concatenate(shards, axis=0)[:N]
    return out.astype(np.float32)
